# revision 16
# baseline (speedup 1.0000x reference)
"""Trainium2 Bass kernel for nn_CubicModelLarge (3-layer cubic-feature MLP).

Strategy: tensor-parallel over the cubic multiplier index i (64 values, 8 per
core).  The cubic expansion is never materialized.  Per layer:

  y[b,o] = W_lin@x + b + sum_t W_sq[o,t] xsq[b,t] + sum_i x[b,i] sum_t W_cu[o,i,t] xsq[b,t]

Rewritten per core c (i in I_c = [8c, 8c+8)):

  H[b,(il,o)] = sum_J F[J,b] * Wcub[J,(il,o)]     (one f32r GEMM, J = 2176 rows)
  y_c[b,o]    = lin[b,o] + sum_il xmac[b,il] * H[b,(il,o)]
  y = AllReduce_c(y_c)

F rows use the sum-square basis: instead of products x_a*x_b, each row is
u = (x_a+x_b)^2/2, built by a selection-SUM matmul on the PE (two 1s per
column) followed by a Square activation on the Scalar engine (PSUM->SBUF).
This removes all DVE tensor_mul product work.  The weight fold
x_a*x_b = u_ab - x_a^2/2 - x_b^2/2 is applied host-side (corrections land on
the d=0 rows, whose value is now 2*x_a^2).

A tiny warm-up AllReduce issues first to absorb collective-init latency.
Final layer partials are summed on the host.
"""

import numpy as np

D = 64
B = 1024
NCORES = 8
I_PER = D // NCORES          # 8
OUTS = (64, 64, 10)
NKCHUNK = 16                 # rotation chunks (d pairs)
NB = B // 128                # 8 batch chunks
INV_SQRT2 = 0.7071067811865476

_CACHE = {}


# ---------------------------------------------------------------- host prep --

def _maps():
    iu, ju = np.triu_indices(D)
    tmap = np.zeros((D, D), np.int64)
    tmap[iu, ju] = np.arange(len(iu))
    tmap[ju, iu] = tmap[iu, ju]
    p = np.arange(128)
    rows_t = np.zeros((NKCHUNK, 128), np.int64)
    for k in range(NKCHUNK):
        d = 2 * k + p // 64
        a = p % 64
        rows_t[k] = tmap[a, (a + d) % D]
    d32_t = tmap[np.arange(D), (np.arange(D) + 32) % D]
    return tmap, rows_t, d32_t


def _u_transform():
    """B.T for the sum-square basis change on the 2176-row F basis.

    Rows 0..2047: rotation products (k = r//128, p = r%128, d = 2k + p//64,
    a = p%64, b = (a+d)%64).  Rows 2048..2111: x rows.  Rows 2112..2175:
    d32 products.  Old row value x_a*x_b = u_r - u_{d0(a)}/4 - u_{d0(b)}/4
    (a != b); d0 rows (a == b): x_a^2 = u_r/2.  d0(a) = row a.
    """
    n = 17 * 128
    Bm = np.zeros((n, n), np.float32)
    for r in range(2048):
        k, p = divmod(r, 128)
        d = 2 * k + p // 64
        a = p % 64
        b = (a + d) % D
        if a == b:
            Bm[r, r] = 0.5
        else:
            Bm[r, r] = 1.0
            Bm[r, a] -= 0.25
            Bm[r, b] -= 0.25
    for r in range(2048, 2048 + D):
        Bm[r, r] = 1.0
    for r in range(2048 + D, n):
        a = r - (2048 + D)
        b = (a + 32) % D
        Bm[r, r] = 1.0
        Bm[r, a] -= 0.25
        Bm[r, b] -= 0.25
    return Bm.T.copy()


def _prep_layer(W, b, out, BT):
    """-> (wcub [NCORES](2176, I_PER*out), wlin [NCORES](65, out))"""
    _, rows_t, d32_t = _maps()
    W_lin = W[:, :D]
    W_sq = W[:, D:D + 2080]
    W_cu = W[:, D + 2080:].reshape(out, D, 2080)

    iu, ju = np.triu_indices(D)
    w2 = np.zeros((out, D, D), np.float32)
    half = np.where(iu == ju, 1.0, 0.5).astype(np.float32)
    w2[:, iu, ju] = W_sq * half
    w2[:, ju, iu] = W_sq * half

    rt = rows_t.reshape(-1)
    wcubs, wlins = [], []
    for core in range(NCORES):
        I = np.arange(core * I_PER, (core + 1) * I_PER)
        M = I_PER * out
        wcub = np.zeros((17 * 128, M), np.float32)
        blk = W_cu[:, I, :][:, :, rt]                       # (out, I_PER, 2048)
        wcub[:2048] = blk.transpose(2, 1, 0).reshape(2048, M)
        w2blk = w2[:, I, :]                                 # (out, I_PER, 64)
        wcub[2048:2048 + D] = w2blk.transpose(2, 1, 0).reshape(D, M)
        d32blk = W_cu[:, I, :][:, :, d32_t] / 2
        wcub[2048 + D:] = d32blk.transpose(2, 1, 0).reshape(D, M)
        wcub = BT @ wcub                                    # sum-square fold
        wcubs.append(np.ascontiguousarray(wcub.astype(np.float16)))

        wl = np.zeros((65, out), np.float32)
        if core == 0:
            wl[:D] = W_lin.T
            wl[D] = b
        wlins.append(wl.astype(np.float16))
    return wcubs, wlins


def _sel_consts():
    """Selection-SUM matrices, concatenated (64, 17*128).

    slot k in 0..15: col p has +1 at row a=(p%64) and +1 at row
    b=(a + 2k + p//64)%64 (coincident at d=0 -> value 2).
    slot 16: first 64 cols build the d32 sums (+1 at a, +1 at (a+32)%64).
    """
    sel = np.zeros((D, 17 * 128), np.float16)
    for k in range(NKCHUNK):
        for p in range(128):
            d = 2 * k + p // 64
            a = p % 64
            sel[a, k * 128 + p] += 1.0
            sel[(a + d) % D, k * 128 + p] += 1.0
    for a in range(D):
        sel[a, NKCHUNK * 128 + a] += 1.0
        sel[(a + 32) % D, NKCHUNK * 128 + a] += 1.0
    return sel


# ------------------------------------------------------------------ builder --

def _build_module():
    import concourse.bacc as bacc
    import concourse.mybir as mybir
    import concourse.tile as tile

    F32 = mybir.dt.float32
    F32R = mybir.dt.float32r
    F16 = mybir.dt.float16
    MULT = mybir.AluOpType.mult
    ADD = mybir.AluOpType.add
    SQUARE = mybir.ActivationFunctionType.Square
    AXIS_X = mybir.AxisListType.X

    nc = bacc.Bacc("TRN2", target_bir_lowering=False, num_devices=NCORES, debug=False)

    x_in = nc.dram_tensor("x", [B, D], F32, kind="ExternalInput")
    wcub_in = [
        nc.dram_tensor(f"wcub{li}", [17 * 128, I_PER * OUTS[li]], F16, kind="ExternalInput")
        for li in range(3)
    ]
    wlin_in = [
        nc.dram_tensor(f"wlin{li}", [65, OUTS[li]], F16, kind="ExternalInput")
        for li in range(3)
    ]
    colsel_in = nc.dram_tensor("colsel", [D, I_PER], F16, kind="ExternalInput")
    out_ext = nc.dram_tensor("out", [B, OUTS[2]], F32, kind="ExternalOutput")

    sel_c = nc.inline_tensor(_sel_consts(), name="selc")
    ident_c = nc.inline_tensor(np.eye(128, dtype=np.float32), name="identc")

    with tile.TileContext(nc) as tc:
        with (
            tc.tile_pool(name="wpool", bufs=2) as wpool,
            tc.tile_pool(name="spool", bufs=1) as spool,
            tc.tile_pool(name="xpool", bufs=2) as xpool,
            tc.tile_pool(name="qpool", bufs=1) as qpool,
            tc.tile_pool(name="ypool", bufs=2) as ypool,
            tc.tile_pool(name="hpool", bufs=3) as hpool,
            tc.tile_pool(name="ps_rep", bufs=2, space="PSUM") as ps_rep,
            tc.tile_pool(name="ps_h", bufs=3, space="PSUM") as ps_h,
            tc.tile_pool(name="ps_small", bufs=3, space="PSUM") as ps_small,
            tc.tile_pool(name="dpool", bufs=2, space="DRAM") as dpool,
        ):
            # ---- warm-up collective: absorb ncfw init + cross-core skew
            warm_src = dpool.tile([128, 4], F32, tag="warm_src")
            warm_dst = dpool.tile([128, 4], F32, tag="warm_dst")
            warm_sb = spool.tile([128, 4], F32, tag="warm_sb")
            nc.vector.memset(warm_sb[:], 0.0)
            nc.sync.dma_start(warm_src[:], warm_sb[:])
            nc.gpsimd.collective_compute(
                "AllReduce",
                ADD,
                replica_groups=[list(range(NCORES))],
                ins=[warm_src.opt()],
                outs=[warm_dst.opt()],
            )

            sel_sb = spool.tile([D, 17 * 128], F16, tag="sel")
            nc.sync.dma_start(sel_sb[:], sel_c.ap())
            ident_sb = spool.tile([128, 128], F32, tag="ident")
            nc.sync.dma_start(ident_sb[:], ident_c.ap())
            colsel_sb = spool.tile([D, I_PER], F16, tag="colsel")
            nc.sync.dma_start(colsel_sb[:], colsel_in.ap())

            HB = 512            # half-batch
            NBH = HB // 128     # 4 chunks per half

            # per-layer weight tiles (split into two k-halves so the first
            # matmuls can start before the whole layer's weights land)
            weights = []
            for li in range(3):
                M = I_PER * OUTS[li]
                wcub_sb = wpool.tile([128, NKCHUNK, M], F16, tag="wcub")
                for kh in range(2):
                    nc.sync.dma_start(
                        wcub_sb[:, kh * 8:(kh + 1) * 8, :],
                        wcub_in[li].ap()[kh * 1024:(kh + 1) * 1024, :]
                        .rearrange("(k p) m -> p k m", p=128),
                    )
                wx_sb = wpool.tile([D, M], F16, tag="wx")
                nc.sync.dma_start(wx_sb[:], wcub_in[li].ap()[2048:2048 + D, :])
                wd32_sb = wpool.tile([D, M], F16, tag="wd32")
                nc.sync.dma_start(wd32_sb[:], wcub_in[li].ap()[2048 + D:, :])
                wlin_sb = wpool.tile([65, OUTS[li]], F16, tag="wlin")
                nc.sync.dma_start(wlin_sb[:], wlin_in[li].ap())
                weights.append((wcub_sb, wx_sb, wd32_sb, wlin_sb))

            # layer-0 x arrives transposed straight from DRAM via the xbar
            x_half = [x_in.ap()[h * HB:(h + 1) * HB, :] for h in range(2)]

            for li in range(3):
                out_l = OUTS[li]
                M = I_PER * out_l
                last = li == 2
                wcub_sb, wx_sb, wd32_sb, wlin_sb = weights[li]
                next_x = [None, None]

                for h in range(2):
                    # -- phase A: xT via DMA-transpose from DRAM + one cast
                    xT32 = xpool.tile([D, HB], F32, tag=f"xT32{h}")
                    nc.sync.dma_start_transpose(xT32[:], x_half[h])
                    xT_sb = xpool.tile([65, HB], F16, tag=f"xT{h}")
                    nc.vector.tensor_copy(xT_sb[0:D, :], xT32[:])
                    nc.vector.memset(xT_sb[D:65, :], 1.0)

                    # d32 rows: u = (x_a + x_{a+32})^2 / 2
                    xd32_sb = xpool.tile([D, HB], F16, tag=f"xd32{h}")
                    rep32 = ps_rep.tile([128, HB], F32, tag="rep")
                    nc.tensor.matmul(
                        rep32[0:D, :], sel_sb[:, NKCHUNK * 128:NKCHUNK * 128 + D],
                        xT_sb[0:D, :], start=True, stop=True,
                    )
                    nc.scalar.activation(
                        xd32_sb[:], rep32[0:D, :], SQUARE, scale=INV_SQRT2
                    )

                    # -- phase B: u chunks via selection-sum matmul + Square
                    xsq = []
                    for k in range(NKCHUNK):
                        rep = ps_rep.tile([128, HB], F32, tag="rep")
                        nc.tensor.matmul(
                            rep[:], sel_sb[:, k * 128:(k + 1) * 128],
                            xT_sb[0:D, :], start=True, stop=True,
                        )
                        xq = qpool.tile([128, HB], F16, tag=f"xsq{k}h{h}")
                        nc.scalar.activation(
                            xq[:], rep[:], SQUARE, scale=INV_SQRT2
                        )
                        xsq.append(xq)

                    # -- phase C
                    y_sb = ypool.tile([128, NBH, out_l], F32, tag=f"y{h}")
                    if not last:
                        for bc in range(NBH):
                            bs = slice(bc * 128, (bc + 1) * 128)
                            h_ps = ps_h.tile([128, M], F32, tag="h")
                            for k in range(NKCHUNK):
                                nc.tensor.matmul(
                                    h_ps[:], xsq[k][:, bs], wcub_sb[:, k, :],
                                    start=(k == 0), stop=False,
                                )
                            nc.tensor.matmul(h_ps[:], xT_sb[0:D, bs], wx_sb[:], start=False, stop=False)
                            nc.tensor.matmul(h_ps[:], xd32_sb[:, bs], wd32_sb[:], start=False, stop=True)

                            lin_ps = ps_small.tile([128, out_l], F32, tag="small")
                            nc.tensor.matmul(lin_ps[:], xT_sb[0:65, bs], wlin_sb[:], start=True, stop=True)
                            xmac_ps = ps_small.tile([128, I_PER], F32, tag="small")
                            nc.tensor.matmul(xmac_ps[:], xT_sb[0:D, bs], colsel_sb[:], start=True, stop=True)
                            xmac_sb = ypool.tile([128, I_PER], F32, tag="xmac")
                            nc.scalar.copy(xmac_sb[:], xmac_ps[:])

                            # tmp[:, :M] = h * xmac (broadcast over o); tmp[:, M:] = lin
                            tmp_sb = hpool.tile([128, M + out_l], F32, tag="tmp")
                            xmac_b = (
                                xmac_sb[:].unsqueeze(2).to_broadcast([128, I_PER, out_l])
                            )
                            nc.vector.tensor_tensor(
                                tmp_sb[:, 0:M].rearrange("p (i o) -> p i o", i=I_PER),
                                h_ps[:].rearrange("p (i o) -> p i o", i=I_PER),
                                xmac_b,
                                op=MULT,
                            )
                            nc.scalar.copy(tmp_sb[:, M:], lin_ps[:])
                            nc.vector.tensor_reduce(
                                y_sb[:, bc, :],
                                tmp_sb[:].rearrange("p (i o) -> p o i", i=I_PER + 1),
                                axis=AXIS_X, op=ADD,
                            )

                        # -- phase D: AllReduce this half
                        y_bounce = dpool.tile([HB, out_l], F32, tag=f"ybounce{h}")
                        y_red = dpool.tile([HB, out_l], F32, tag=f"yred{h}")
                        nc.sync.dma_start(
                            y_bounce[:].rearrange("(bc p) o -> p bc o", p=128), y_sb[:]
                        )
                        nc.gpsimd.collective_compute(
                            "AllReduce",
                            ADD,
                            replica_groups=[list(range(NCORES))],
                            ins=[y_bounce.opt()],
                            outs=[y_red.opt()],
                        )
                        next_x[h] = y_red[:]
                    else:
                        # layer 2: stationary-W GEMM, transpose, MAC
                        h_ps = ps_h.tile([M, HB], F32, tag="h")
                        for k in range(NKCHUNK):
                            nc.tensor.matmul(
                                h_ps[:], wcub_sb[:, k, :], xsq[k][:],
                                start=(k == 0), stop=False,
                            )
                        nc.tensor.matmul(h_ps[:], wx_sb[:], xT_sb[0:D, :], start=False, stop=False)
                        nc.tensor.matmul(h_ps[:], wd32_sb[:], xd32_sb[:], start=False, stop=True)
                        h2_sb = ypool.tile([M, HB], F32, tag=f"h2{h}")
                        nc.vector.tensor_copy(h2_sb[:], h_ps[:])

                        for bc in range(NBH):
                            bs = slice(bc * 128, (bc + 1) * 128)
                            h2t_ps = ps_small.tile([128, M], F32, tag="small")
                            nc.tensor.transpose(h2t_ps[:], h2_sb[:, bs], ident_sb[0:M, 0:M])

                            lin_ps = ps_small.tile([128, out_l], F32, tag="small")
                            nc.tensor.matmul(lin_ps[:], xT_sb[0:65, bs], wlin_sb[:], start=True, stop=True)
                            xmac_ps = ps_small.tile([128, I_PER], F32, tag="small")
                            nc.tensor.matmul(xmac_ps[:], xT_sb[0:D, bs], colsel_sb[:], start=True, stop=True)
                            xmac_sb = ypool.tile([128, I_PER], F32, tag="xmac")
                            nc.scalar.copy(xmac_sb[:], xmac_ps[:])

                            tmp_sb = hpool.tile([128, M + out_l], F32, tag="tmp2")
                            xmac_b = (
                                xmac_sb[:].unsqueeze(2).to_broadcast([128, I_PER, out_l])
                            )
                            nc.vector.tensor_tensor(
                                tmp_sb[:, 0:M].rearrange("p (i o) -> p i o", i=I_PER),
                                h2t_ps[:].rearrange("p (i o) -> p i o", i=I_PER),
                                xmac_b,
                                op=MULT,
                            )
                            nc.scalar.copy(tmp_sb[:, M:], lin_ps[:])
                            nc.vector.tensor_reduce(
                                y_sb[:, bc, :],
                                tmp_sb[:].rearrange("p (i o) -> p o i", i=I_PER + 1),
                                axis=AXIS_X, op=ADD,
                            )

                        nc.sync.dma_start(
                            out_ext.ap()[h * HB:(h + 1) * HB, :]
                            .rearrange("(bc p) o -> p bc o", p=128),
                            y_sb[:],
                        )

                if not last:
                    x_half = next_x

    nc.compile()
    return nc


# ------------------------------------------------------------------- runner --

def kernel(x, W0, b0, W1, b1, W2, b2):
    from concourse.bass_utils import run_bass_kernel_spmd

    if "nc" not in _CACHE:
        _CACHE["nc"] = _build_module()
    nc = _CACHE["nc"]

    x = np.ascontiguousarray(np.asarray(x, np.float32))
    Ws = [np.asarray(W, np.float32) for W in (W0, W1, W2)]
    bs = [np.asarray(b_, np.float32) for b_ in (b0, b1, b2)]

    BT = _u_transform()
    wcubs, wlins = {}, {}
    for li in range(3):
        wcubs[li], wlins[li] = _prep_layer(Ws[li], bs[li], OUTS[li], BT)

    in_maps = []
    for core in range(NCORES):
        I = np.arange(core * I_PER, (core + 1) * I_PER)
        colsel = np.zeros((D, I_PER), np.float16)
        colsel[I, np.arange(I_PER)] = 1.0
        m = {"x": x, "colsel": colsel}
        for li in range(3):
            m[f"wcub{li}"] = wcubs[li][core]
            m[f"wlin{li}"] = wlins[li][core]
        in_maps.append(m)

    res = run_bass_kernel_spmd(nc, in_maps, core_ids=list(range(NCORES)))
    out = np.zeros((B, OUTS[2]), np.float32)
    for core in range(NCORES):
        out += res.results[core]["out"]
    return out


# revision 19
# speedup vs baseline: 1.5121x; 1.5121x over previous
"""Trainium2 Bass kernel for nn_CubicModelLarge (3-layer cubic-feature MLP).

Strategy: tensor-parallel over the cubic multiplier index i (64 values, 8 per
core).  The cubic expansion is never materialized.  Per layer:

  y[b,o] = W_lin@x + b + sum_t W_sq[o,t] xsq[b,t] + sum_i x[b,i] sum_t W_cu[o,i,t] xsq[b,t]

Rewritten per core c (i in I_c = [8c, 8c+8)):

  H[b,(il,o)] = sum_J F[J,b] * Wcub[J,(il,o)]     (one f32r GEMM, J = 2176 rows)
  y_c[b,o]    = lin[b,o] + sum_il xmac[b,il] * H[b,(il,o)]
  y = AllReduce_c(y_c)

F rows use the sum-square basis: instead of products x_a*x_b, each row is
u = (x_a+x_b)^2/2, built by a selection-SUM matmul on the PE (two 1s per
column) followed by a Square activation on the Scalar engine (PSUM->SBUF).
This removes all DVE tensor_mul product work.  The weight fold
x_a*x_b = u_ab - x_a^2/2 - x_b^2/2 is applied host-side (corrections land on
the d=0 rows, whose value is now 2*x_a^2).

A tiny warm-up AllReduce issues first to absorb collective-init latency.
Final layer partials are summed on the host.
"""

import numpy as np

D = 64
B = 1024
NCORES = 8
I_PER = D // NCORES          # 8
OUTS = (64, 64, 10)
NKCHUNK = 16                 # rotation chunks (d pairs)
NB = B // 128                # 8 batch chunks
INV_SQRT2 = 0.7071067811865476

_CACHE = {}


# ---------------------------------------------------------------- host prep --

def _maps():
    iu, ju = np.triu_indices(D)
    tmap = np.zeros((D, D), np.int64)
    tmap[iu, ju] = np.arange(len(iu))
    tmap[ju, iu] = tmap[iu, ju]
    p = np.arange(128)
    rows_t = np.zeros((NKCHUNK, 128), np.int64)
    for k in range(NKCHUNK):
        d = 2 * k + p // 64
        a = p % 64
        rows_t[k] = tmap[a, (a + d) % D]
    d32_t = tmap[np.arange(D), (np.arange(D) + 32) % D]
    return tmap, rows_t, d32_t


def _u_transform():
    """B.T for the sum-square basis change on the 2176-row F basis.

    Rows 0..2047: rotation products (k = r//128, p = r%128, d = 2k + p//64,
    a = p%64, b = (a+d)%64).  Rows 2048..2111: x rows.  Rows 2112..2175:
    d32 products.  Old row value x_a*x_b = u_r - u_{d0(a)}/4 - u_{d0(b)}/4
    (a != b); d0 rows (a == b): x_a^2 = u_r/2.  d0(a) = row a.
    """
    n = 17 * 128
    Bm = np.zeros((n, n), np.float32)
    for r in range(2048):
        k, p = divmod(r, 128)
        d = 2 * k + p // 64
        a = p % 64
        b = (a + d) % D
        if a == b:
            Bm[r, r] = 0.5
        else:
            Bm[r, r] = 1.0
            Bm[r, a] -= 0.25
            Bm[r, b] -= 0.25
    for r in range(2048, 2048 + D):
        Bm[r, r] = 1.0
    for r in range(2048 + D, n):
        a = r - (2048 + D)
        b = (a + 32) % D
        Bm[r, r] = 1.0
        Bm[r, a] -= 0.25
        Bm[r, b] -= 0.25
    return Bm.T.copy()


def _prep_layer(W, b, out, BT):
    """-> (wcub [NCORES](2176, I_PER*out), wlin [NCORES](65, out))"""
    _, rows_t, d32_t = _maps()
    W_lin = W[:, :D]
    W_sq = W[:, D:D + 2080]
    W_cu = W[:, D + 2080:].reshape(out, D, 2080)

    iu, ju = np.triu_indices(D)
    w2 = np.zeros((out, D, D), np.float32)
    half = np.where(iu == ju, 1.0, 0.5).astype(np.float32)
    w2[:, iu, ju] = W_sq * half
    w2[:, ju, iu] = W_sq * half

    rt = rows_t.reshape(-1)
    wcubs, wlins = [], []
    for core in range(NCORES):
        I = np.arange(core * I_PER, (core + 1) * I_PER)
        M = I_PER * out
        wcub = np.zeros((17 * 128, M), np.float32)
        blk = W_cu[:, I, :][:, :, rt]                       # (out, I_PER, 2048)
        wcub[:2048] = blk.transpose(2, 1, 0).reshape(2048, M)
        w2blk = w2[:, I, :]                                 # (out, I_PER, 64)
        wcub[2048:2048 + D] = w2blk.transpose(2, 1, 0).reshape(D, M)
        d32blk = W_cu[:, I, :][:, :, d32_t] / 2
        wcub[2048 + D:] = d32blk.transpose(2, 1, 0).reshape(D, M)
        wcub = BT @ wcub                                    # sum-square fold
        wcubs.append(np.ascontiguousarray(wcub.astype(np.float16)))

        wl = np.zeros((65, out), np.float32)
        if core == 0:
            wl[:D] = W_lin.T
            wl[D] = b
        wlins.append(wl.astype(np.float16))
    return wcubs, wlins


def _sel_consts():
    """Selection-SUM matrices, concatenated (64, 17*128).

    slot k in 0..15: col p has +1 at row a=(p%64) and +1 at row
    b=(a + 2k + p//64)%64 (coincident at d=0 -> value 2).
    slot 16: first 64 cols build the d32 sums (+1 at a, +1 at (a+32)%64).
    """
    sel = np.zeros((D, 17 * 128), np.float16)
    for k in range(NKCHUNK):
        for p in range(128):
            d = 2 * k + p // 64
            a = p % 64
            sel[a, k * 128 + p] += 1.0
            sel[(a + d) % D, k * 128 + p] += 1.0
    for a in range(D):
        sel[a, NKCHUNK * 128 + a] += 1.0
        sel[(a + 32) % D, NKCHUNK * 128 + a] += 1.0
    return sel


# ------------------------------------------------------------------ builder --

def _build_module():
    import concourse.bacc as bacc
    import concourse.mybir as mybir
    import concourse.tile as tile

    F32 = mybir.dt.float32
    F32R = mybir.dt.float32r
    F16 = mybir.dt.float16
    MULT = mybir.AluOpType.mult
    ADD = mybir.AluOpType.add
    SQUARE = mybir.ActivationFunctionType.Square
    AXIS_X = mybir.AxisListType.X

    nc = bacc.Bacc("TRN2", target_bir_lowering=False, num_devices=NCORES, debug=False)

    x_in = nc.dram_tensor("x", [B, D], F32, kind="ExternalInput")
    wcub_in = [
        nc.dram_tensor(f"wcub{li}", [17 * 128, I_PER * OUTS[li]], F16, kind="ExternalInput")
        for li in range(3)
    ]
    wlin_in = [
        nc.dram_tensor(f"wlin{li}", [65, OUTS[li]], F16, kind="ExternalInput")
        for li in range(3)
    ]
    colsel_in = nc.dram_tensor("colsel", [D, I_PER], F16, kind="ExternalInput")
    out_ext = nc.dram_tensor("out", [B, OUTS[2]], F32, kind="ExternalOutput")

    sel_c = nc.inline_tensor(_sel_consts(), name="selc")
    ident_c = nc.inline_tensor(np.eye(128, dtype=np.float32), name="identc")

    with tile.TileContext(nc) as tc:
        with (
            tc.tile_pool(name="wpool", bufs=2) as wpool,
            tc.tile_pool(name="spool", bufs=1) as spool,
            tc.tile_pool(name="xpool", bufs=2) as xpool,
            tc.tile_pool(name="qpool", bufs=1) as qpool,
            tc.tile_pool(name="ypool", bufs=2) as ypool,
            tc.tile_pool(name="hpool", bufs=3) as hpool,
            tc.tile_pool(name="ps_rep", bufs=2, space="PSUM") as ps_rep,
            tc.tile_pool(name="ps_h", bufs=3, space="PSUM") as ps_h,
            tc.tile_pool(name="ps_small", bufs=3, space="PSUM") as ps_small,
            tc.tile_pool(name="dpool", bufs=2, space="DRAM") as dpool,
        ):
            # ---- warm-up collective: absorb ncfw init + cross-core skew
            warm_src = dpool.tile([128, 4], F32, tag="warm_src")
            warm_dst = dpool.tile([128, 4], F32, tag="warm_dst")
            warm_sb = spool.tile([128, 4], F32, tag="warm_sb")
            nc.vector.memset(warm_sb[:], 0.0)
            nc.sync.dma_start(warm_src[:], warm_sb[:])
            nc.gpsimd.collective_compute(
                "AllReduce",
                ADD,
                replica_groups=[list(range(NCORES))],
                ins=[warm_src.opt()],
                outs=[warm_dst.opt()],
            )

            sel_sb = spool.tile([D, 17 * 128], F16, tag="sel")
            nc.sync.dma_start(sel_sb[:], sel_c.ap())
            ident_sb = spool.tile([128, 128], F32, tag="ident")
            nc.sync.dma_start(ident_sb[:], ident_c.ap())
            colsel_sb = spool.tile([D, I_PER], F16, tag="colsel")
            nc.sync.dma_start(colsel_sb[:], colsel_in.ap())

            HB = 512            # half-batch
            NBH = HB // 128     # 4 chunks per half

            # per-layer weight tiles (split into two k-halves so the first
            # matmuls can start before the whole layer's weights land)
            weights = []
            for li in range(3):
                M = I_PER * OUTS[li]
                wcub_sb = wpool.tile([128, NKCHUNK, M], F16, tag="wcub")
                for kh in range(2):
                    nc.sync.dma_start(
                        wcub_sb[:, kh * 8:(kh + 1) * 8, :],
                        wcub_in[li].ap()[kh * 1024:(kh + 1) * 1024, :]
                        .rearrange("(k p) m -> p k m", p=128),
                    )
                wx_sb = wpool.tile([D, M], F16, tag="wx")
                nc.sync.dma_start(wx_sb[:], wcub_in[li].ap()[2048:2048 + D, :])
                wd32_sb = wpool.tile([D, M], F16, tag="wd32")
                nc.sync.dma_start(wd32_sb[:], wcub_in[li].ap()[2048 + D:, :])
                wlin_sb = wpool.tile([65, OUTS[li]], F16, tag="wlin")
                nc.sync.dma_start(wlin_sb[:], wlin_in[li].ap())
                weights.append((wcub_sb, wx_sb, wd32_sb, wlin_sb))

            # x tiles for layer 0, both halves, straight from the input
            x_half = []
            for h in range(2):
                xs = xpool.tile([128, NBH, D], F32, tag=f"x{h}")
                nc.sync.dma_start(
                    xs[:],
                    x_in.ap()[h * HB:(h + 1) * HB, :]
                    .rearrange("(bc p) f -> p bc f", p=128),
                )
                x_half.append(xs)

            for li in range(3):
                out_l = OUTS[li]
                M = I_PER * out_l
                last = li == 2
                wcub_sb, wx_sb, wd32_sb, wlin_sb = weights[li]
                next_x = [None, None]

                for h in range(2):
                    x_sb = x_half[h]

                    # -- phase A: xT via PE transposes + cast
                    xT_sb = xpool.tile([65, HB], F16, tag=f"xT{h}")
                    for bc in range(NBH):
                        xTp = ps_small.tile([D, 128], F32, tag="small")
                        nc.tensor.transpose(xTp[:], x_sb[:, bc, :], ident_sb[:])
                        nc.scalar.copy(xT_sb[0:D, bc * 128:(bc + 1) * 128], xTp[:])
                    nc.vector.memset(xT_sb[D:65, :], 1.0)

                    # d32 rows: u = (x_a + x_{a+32})^2 / 2
                    xd32_sb = xpool.tile([D, HB], F16, tag=f"xd32{h}")
                    rep32 = ps_rep.tile([128, HB], F32, tag="rep")
                    nc.tensor.matmul(
                        rep32[0:D, :], sel_sb[:, NKCHUNK * 128:NKCHUNK * 128 + D],
                        xT_sb[0:D, :], start=True, stop=True,
                    )
                    nc.scalar.activation(
                        xd32_sb[:], rep32[0:D, :], SQUARE, scale=INV_SQRT2
                    )

                    # -- phase B: u chunks via selection-sum matmul + Square
                    xsq = []
                    for k in range(NKCHUNK):
                        rep = ps_rep.tile([128, HB], F32, tag="rep")
                        nc.tensor.matmul(
                            rep[:], sel_sb[:, k * 128:(k + 1) * 128],
                            xT_sb[0:D, :], start=True, stop=True,
                        )
                        xq = qpool.tile([128, HB], F16, tag=f"xsq{k}h{h}")
                        nc.scalar.activation(
                            xq[:], rep[:], SQUARE, scale=INV_SQRT2
                        )
                        xsq.append(xq)

                    # -- phase C
                    y_sb = ypool.tile([128, NBH, out_l], F32, tag=f"y{h}")
                    if not last:
                        for bc in range(NBH):
                            bs = slice(bc * 128, (bc + 1) * 128)
                            h_ps = ps_h.tile([128, M], F32, tag="h")
                            for k in range(NKCHUNK):
                                nc.tensor.matmul(
                                    h_ps[:], xsq[k][:, bs], wcub_sb[:, k, :],
                                    start=(k == 0), stop=False,
                                )
                            nc.tensor.matmul(h_ps[:], xT_sb[0:D, bs], wx_sb[:], start=False, stop=False)
                            nc.tensor.matmul(h_ps[:], xd32_sb[:, bs], wd32_sb[:], start=False, stop=True)

                            lin_ps = ps_small.tile([128, out_l], F32, tag="small")
                            nc.tensor.matmul(lin_ps[:], xT_sb[0:65, bs], wlin_sb[:], start=True, stop=True)
                            xmac_ps = ps_small.tile([128, I_PER], F32, tag="small")
                            nc.tensor.matmul(xmac_ps[:], xT_sb[0:D, bs], colsel_sb[:], start=True, stop=True)
                            xmac_sb = ypool.tile([128, I_PER], F32, tag="xmac")
                            nc.scalar.copy(xmac_sb[:], xmac_ps[:])

                            # tmp[:, :M] = h * xmac (broadcast over o); tmp[:, M:] = lin
                            tmp_sb = hpool.tile([128, M + out_l], F32, tag="tmp")
                            xmac_b = (
                                xmac_sb[:].unsqueeze(2).to_broadcast([128, I_PER, out_l])
                            )
                            nc.vector.tensor_tensor(
                                tmp_sb[:, 0:M].rearrange("p (i o) -> p i o", i=I_PER),
                                h_ps[:].rearrange("p (i o) -> p i o", i=I_PER),
                                xmac_b,
                                op=MULT,
                            )
                            nc.scalar.copy(tmp_sb[:, M:], lin_ps[:])
                            nc.vector.tensor_reduce(
                                y_sb[:, bc, :],
                                tmp_sb[:].rearrange("p (i o) -> p o i", i=I_PER + 1),
                                axis=AXIS_X, op=ADD,
                            )

                        # -- phase D: AllReduce this half
                        y_bounce = dpool.tile([HB, out_l], F32, tag=f"ybounce{h}")
                        y_red = dpool.tile([HB, out_l], F32, tag=f"yred{h}")
                        nc.sync.dma_start(
                            y_bounce[:].rearrange("(bc p) o -> p bc o", p=128), y_sb[:]
                        )
                        nc.gpsimd.collective_compute(
                            "AllReduce",
                            ADD,
                            replica_groups=[list(range(NCORES))],
                            ins=[y_bounce.opt()],
                            outs=[y_red.opt()],
                        )
                        xs = xpool.tile([128, NBH, D], F32, tag=f"x{h}")
                        nc.sync.dma_start(
                            xs[:], y_red[:].rearrange("(bc p) f -> p bc f", p=128)
                        )
                        next_x[h] = xs
                    else:
                        # layer 2: stationary-W GEMM, transpose, MAC
                        h_ps = ps_h.tile([M, HB], F32, tag="h")
                        for k in range(NKCHUNK):
                            nc.tensor.matmul(
                                h_ps[:], wcub_sb[:, k, :], xsq[k][:],
                                start=(k == 0), stop=False,
                            )
                        nc.tensor.matmul(h_ps[:], wx_sb[:], xT_sb[0:D, :], start=False, stop=False)
                        nc.tensor.matmul(h_ps[:], wd32_sb[:], xd32_sb[:], start=False, stop=True)
                        h2_sb = ypool.tile([M, HB], F32, tag=f"h2{h}")
                        nc.vector.tensor_copy(h2_sb[:], h_ps[:])

                        for bc in range(NBH):
                            bs = slice(bc * 128, (bc + 1) * 128)
                            h2t_ps = ps_small.tile([128, M], F32, tag="small")
                            nc.tensor.transpose(h2t_ps[:], h2_sb[:, bs], ident_sb[0:M, 0:M])

                            lin_ps = ps_small.tile([128, out_l], F32, tag="small")
                            nc.tensor.matmul(lin_ps[:], xT_sb[0:65, bs], wlin_sb[:], start=True, stop=True)
                            xmac_ps = ps_small.tile([128, I_PER], F32, tag="small")
                            nc.tensor.matmul(xmac_ps[:], xT_sb[0:D, bs], colsel_sb[:], start=True, stop=True)
                            xmac_sb = ypool.tile([128, I_PER], F32, tag="xmac")
                            nc.scalar.copy(xmac_sb[:], xmac_ps[:])

                            tmp_sb = hpool.tile([128, M + out_l], F32, tag="tmp2")
                            xmac_b = (
                                xmac_sb[:].unsqueeze(2).to_broadcast([128, I_PER, out_l])
                            )
                            nc.vector.tensor_tensor(
                                tmp_sb[:, 0:M].rearrange("p (i o) -> p i o", i=I_PER),
                                h2t_ps[:].rearrange("p (i o) -> p i o", i=I_PER),
                                xmac_b,
                                op=MULT,
                            )
                            nc.scalar.copy(tmp_sb[:, M:], lin_ps[:])
                            nc.vector.tensor_reduce(
                                y_sb[:, bc, :],
                                tmp_sb[:].rearrange("p (i o) -> p o i", i=I_PER + 1),
                                axis=AXIS_X, op=ADD,
                            )

                        nc.sync.dma_start(
                            out_ext.ap()[h * HB:(h + 1) * HB, :]
                            .rearrange("(bc p) o -> p bc o", p=128),
                            y_sb[:],
                        )

                if not last:
                    x_half = next_x

    nc.compile()
    return nc


# ------------------------------------------------------------------- runner --

def kernel(x, W0, b0, W1, b1, W2, b2):
    from concourse.bass_utils import run_bass_kernel_spmd

    if "nc" not in _CACHE:
        _CACHE["nc"] = _build_module()
    nc = _CACHE["nc"]

    x = np.ascontiguousarray(np.asarray(x, np.float32))
    Ws = [np.asarray(W, np.float32) for W in (W0, W1, W2)]
    bs = [np.asarray(b_, np.float32) for b_ in (b0, b1, b2)]

    BT = _u_transform()
    wcubs, wlins = {}, {}
    for li in range(3):
        wcubs[li], wlins[li] = _prep_layer(Ws[li], bs[li], OUTS[li], BT)

    in_maps = []
    for core in range(NCORES):
        I = np.arange(core * I_PER, (core + 1) * I_PER)
        colsel = np.zeros((D, I_PER), np.float16)
        colsel[I, np.arange(I_PER)] = 1.0
        m = {"x": x, "colsel": colsel}
        for li in range(3):
            m[f"wcub{li}"] = wcubs[li][core]
            m[f"wlin{li}"] = wlins[li][core]
        in_maps.append(m)

    res = run_bass_kernel_spmd(nc, in_maps, core_ids=list(range(NCORES)))
    out = np.zeros((B, OUTS[2]), np.float32)
    for core in range(NCORES):
        out += res.results[core]["out"]
    return out


# revision 20
# speedup vs baseline: 1.9301x; 1.2764x over previous
"""Trainium2 Bass kernel for nn_CubicModelLarge (3-layer cubic-feature MLP).

Tensor-parallel over the cubic min-index p (64 values, 8 per core; core c,
slot s -> p = 8s + c).  Monomial folding: each cubic monomial x_p x_q x_r
(p<=q<=r) is accumulated once into block p, contracting the triu pair basis

  u_(q,r) = (x_q + x_r)^2 / 2   (q < r, q-major tail order)
  sq_q    = x_q^2
  x_k     = x_k                 (carries the folded quadratic weights)

Block p only needs rows with q >= p, so chunk-level tail skipping cuts the
streamed GEMM columns ~2x vs the unfolded basis (slot-prefix packing keeps
the schedule SPMD-uniform; shorter-tail cores just carry zero weights).

Per core, per layer:
  H[b,(s,o)] = sum_rows F[row,b] * Wfold[row,(s,o)]    (fp16 GEMM, fp32 PSUM)
  y_c[b,o]   = lin[b,o] + sum_s xmac[b,s] * H[b,(s,o)]  (DVE mult+reduce)
  y          = AllReduce_c(y_c)

u rows are built by a selection-SUM matmul on the PE (two 1s per column)
followed by a Square activation on the Scalar engine (PSUM->SBUF fp16).
A tiny warm-up AllReduce issues first to absorb collective-init latency.
Final layer partials are summed on the host.
"""

import numpy as np

D = 64
B = 1024
NCORES = 8
NSLOT = D // NCORES          # 8 slots (i-values) per core
OUTS = (64, 64, 10)
NUC = 16                     # u chunks (2016 rows + pad)
NCHUNK = 17                  # + 1 [sq; x] chunk
INV_SQRT2 = 0.7071067811865476

# slot s covers p in [8s, 8s+8); its tail starts at u-row off(8s)
_OFF = [q * (127 - q) // 2 for q in range(D)]
SLOT_START = [_OFF[8 * s] // 128 for s in range(NSLOT)]      # [0,3,6,9,11,13,14,15]
NSLOTS_AT = [sum(1 for s in range(NSLOT) if SLOT_START[s] <= c) for c in range(NUC)] + [NSLOT]
CHUNK_ORDER = [15, 16, 14, 13, 12, 11, 10, 9, 8, 7, 6, 5, 4, 3, 2, 1, 0]

_CACHE = {}


# ---------------------------------------------------------------- host prep --

def _pair_rows():
    """u-row index map: rows 0..2015 are pairs (q<r) q-major."""
    Q = np.zeros(2016, np.int64)
    R = np.zeros(2016, np.int64)
    for q in range(D):
        o = _OFF[q]
        n = 63 - q
        Q[o:o + n] = q
        R[o:o + n] = np.arange(q + 1, D)
    return Q, R


def _fold_blocks(W, out):
    """-> G [17*128, 64(p), out] folded coefficients per block p."""
    W_sq = W[:, D:D + 2080]
    W_cu = W[:, D + 2080:].reshape(out, D, 2080)
    iu, ju = np.triu_indices(D)

    # T3[p, q, r, out]: sum of W_cu[o, i, (j,k)] over placements, sorted triple
    T3 = np.zeros((D, D, D, out), np.float32)
    I = np.repeat(np.arange(D), 2080)
    J = np.tile(iu, D)
    K = np.tile(ju, D)
    S = np.sort(np.stack([I, J, K]), axis=0)
    V = W_cu.transpose(1, 2, 0).reshape(-1, out)
    np.add.at(T3, (S[0], S[1], S[2]), V)

    Q, R = _pair_rows()
    G = np.zeros((NCHUNK * 128, D, out), np.float32)
    # u-rows: G[row(q,r), p] = T3[p, q, r]  (zero when q < p by construction)
    G[:2016] = T3[:, Q, R, :].transpose(1, 0, 2)
    # sq-rows: diag cubic minus u-substitution corrections
    rowsum = T3.sum(axis=2)                     # [p, q, out] : sum_r T3[p,q,r]
    colsum = T3.sum(axis=1)                     # [p, r, out] : sum_q T3[p,q,r]
    diag = T3[:, np.arange(D), np.arange(D), :]  # [p, q, out]
    sqco = diag - 0.5 * (rowsum + colsum - 2 * diag)
    G[2048:2048 + D] = sqco.transpose(1, 0, 2)
    # x-rows: folded quadratic, pairs with min = p
    tmap = np.zeros((D, D), np.int64)
    tmap[iu, ju] = np.arange(2080)
    tmap[ju, iu] = tmap[iu, ju]
    Wsym = W_sq[:, tmap]                        # [out, p, k]
    mask = (np.arange(D)[None, :] >= np.arange(D)[:, None]).astype(np.float32)
    G[2112:2112 + D] = (Wsym * mask[None]).transpose(2, 1, 0)
    return G


def _prep_layer(W, b, out):
    """-> (wcub [NCORES](17*128, NSLOT*out) fp16, wlin [NCORES](65, out) fp16)"""
    G = _fold_blocks(W, out)
    wcubs, wlins = [], []
    for core in range(NCORES):
        wcub = np.zeros((NCHUNK * 128, NSLOT * out), np.float32)
        for s in range(NSLOT):
            wcub[:, s * out:(s + 1) * out] = G[:, 8 * s + core, :]
        wcubs.append(np.ascontiguousarray(wcub.astype(np.float16)))
        wl = np.zeros((65, out), np.float32)
        if core == 0:
            wl[:D] = W[:, :D].T
            wl[D] = b
        wlins.append(wl.astype(np.float16))
    return wcubs, wlins


def _sel_consts():
    """Selection-SUM matrices (64, 17*128), fp16.

    chunk c<16, col p: +1 at rows Q[128c+p], R[128c+p] (zero cols past 2016).
    chunk 16: col a (a<64): +1 at row a (builds x_a, squared to x_a^2).
    """
    Q, R = _pair_rows()
    sel = np.zeros((D, NCHUNK * 128), np.float16)
    for rho in range(2016):
        sel[Q[rho], rho] += 1.0
        sel[R[rho], rho] += 1.0
    for a in range(D):
        sel[a, NUC * 128 + a] += 1.0
    return sel


# ------------------------------------------------------------------ builder --

def _build_module():
    import concourse.bacc as bacc
    import concourse.mybir as mybir
    import concourse.tile as tile

    F32 = mybir.dt.float32
    F16 = mybir.dt.float16
    MULT = mybir.AluOpType.mult
    ADD = mybir.AluOpType.add
    SQUARE = mybir.ActivationFunctionType.Square
    AXIS_X = mybir.AxisListType.X

    nc = bacc.Bacc("TRN2", target_bir_lowering=False, num_devices=NCORES, debug=False)

    x_in = nc.dram_tensor("x", [B, D], F32, kind="ExternalInput")
    wcub_in = [
        nc.dram_tensor(f"wcub{li}", [NCHUNK * 128, NSLOT * OUTS[li]], F16, kind="ExternalInput")
        for li in range(3)
    ]
    wlin_in = [
        nc.dram_tensor(f"wlin{li}", [65, OUTS[li]], F16, kind="ExternalInput")
        for li in range(3)
    ]
    colsel_in = nc.dram_tensor("colsel", [D, NSLOT], F16, kind="ExternalInput")
    out_ext = nc.dram_tensor("out", [B, OUTS[2]], F32, kind="ExternalOutput")

    sel_c = nc.inline_tensor(_sel_consts(), name="selc")
    ident_c = nc.inline_tensor(np.eye(128, dtype=np.float32), name="identc")

    with tile.TileContext(nc) as tc:
        with (
            tc.tile_pool(name="wpool", bufs=2) as wpool,
            tc.tile_pool(name="spool", bufs=1) as spool,
            tc.tile_pool(name="xpool", bufs=2) as xpool,
            tc.tile_pool(name="qpool", bufs=1) as qpool,
            tc.tile_pool(name="ypool", bufs=2) as ypool,
            tc.tile_pool(name="hpool", bufs=3) as hpool,
            tc.tile_pool(name="ps_rep", bufs=2, space="PSUM") as ps_rep,
            tc.tile_pool(name="ps_h", bufs=3, space="PSUM") as ps_h,
            tc.tile_pool(name="ps_small", bufs=3, space="PSUM") as ps_small,
            tc.tile_pool(name="dpool", bufs=2, space="DRAM") as dpool,
        ):
            # ---- warm-up collective: absorb ncfw init + cross-core skew
            warm_src = dpool.tile([128, 4], F32, tag="warm_src")
            warm_dst = dpool.tile([128, 4], F32, tag="warm_dst")
            warm_sb = spool.tile([128, 4], F32, tag="warm_sb")
            nc.vector.memset(warm_sb[:], 0.0)
            nc.sync.dma_start(warm_src[:], warm_sb[:])
            nc.gpsimd.collective_compute(
                "AllReduce",
                ADD,
                replica_groups=[list(range(NCORES))],
                ins=[warm_src.opt()],
                outs=[warm_dst.opt()],
            )

            sel_sb = spool.tile([D, NCHUNK * 128], F16, tag="sel")
            nc.sync.dma_start(sel_sb[:], sel_c.ap())
            ident_sb = spool.tile([128, 128], F32, tag="ident")
            nc.sync.dma_start(ident_sb[:], ident_c.ap())
            colsel_sb = spool.tile([D, NSLOT], F16, tag="colsel")
            nc.sync.dma_start(colsel_sb[:], colsel_in.ap())

            HB = 512            # half-batch
            NBH = HB // 128     # 4 chunks per half

            # per-layer weight tiles; only the active slot-prefix per chunk
            weights = []
            for li in range(3):
                out_l = OUTS[li]
                M = NSLOT * out_l
                wcub_sb = wpool.tile([128, NCHUNK, M], F16, tag="wcub")
                for c in range(NCHUNK):
                    w = out_l * NSLOTS_AT[c]
                    nc.sync.dma_start(
                        wcub_sb[:, c, 0:w],
                        wcub_in[li].ap()[c * 128:(c + 1) * 128, 0:w],
                    )
                wlin_sb = wpool.tile([65, out_l], F16, tag="wlin")
                nc.sync.dma_start(wlin_sb[:], wlin_in[li].ap())
                weights.append((wcub_sb, wlin_sb))

            # x tiles for layer 0, both halves, straight from the input
            x_half = []
            for h in range(2):
                xs = xpool.tile([128, NBH, D], F32, tag=f"x{h}")
                nc.sync.dma_start(
                    xs[:],
                    x_in.ap()[h * HB:(h + 1) * HB, :]
                    .rearrange("(bc p) f -> p bc f", p=128),
                )
                x_half.append(xs)

            for li in range(3):
                out_l = OUTS[li]
                M = NSLOT * out_l
                last = li == 2
                wcub_sb, wlin_sb = weights[li]
                next_x = [None, None]

                for h in range(2):
                    x_sb = x_half[h]

                    # -- phase A: xT via PE transposes + cast
                    xT_sb = xpool.tile([65, HB], F16, tag=f"xT{h}")
                    for bc in range(NBH):
                        xTp = ps_small.tile([D, 128], F32, tag="small")
                        nc.tensor.transpose(xTp[:], x_sb[:, bc, :], ident_sb[:])
                        nc.scalar.copy(xT_sb[0:D, bc * 128:(bc + 1) * 128], xTp[:])
                    nc.vector.memset(xT_sb[D:65, :], 1.0)

                    # -- phase B: u chunks (sel-sum matmul + Square); chunk 16 = [sq; x]
                    xsq = []
                    for c in range(NUC):
                        rep = ps_rep.tile([128, HB], F32, tag="rep")
                        nc.tensor.matmul(
                            rep[:], sel_sb[:, c * 128:(c + 1) * 128],
                            xT_sb[0:D, :], start=True, stop=True,
                        )
                        xq = qpool.tile([128, HB], F16, tag=f"xsq{c}h{h}")
                        nc.scalar.activation(xq[:], rep[:], SQUARE, scale=INV_SQRT2)
                        xsq.append(xq)
                    rep16 = ps_rep.tile([128, HB], F32, tag="rep")
                    nc.tensor.matmul(
                        rep16[0:D, :], sel_sb[:, NUC * 128:NUC * 128 + D],
                        xT_sb[0:D, :], start=True, stop=True,
                    )
                    xq16 = qpool.tile([128, HB], F16, tag=f"xsq16h{h}")
                    nc.scalar.activation(xq16[0:D, :], rep16[0:D, :], SQUARE, scale=1.0)
                    nc.vector.tensor_copy(xq16[D:128, :], xT_sb[0:D, :])
                    xsq.append(xq16)

                    # -- phase C
                    y_sb = ypool.tile([128, NBH, out_l], F32, tag=f"y{h}")
                    for bc in range(NBH):
                        bs = slice(bc * 128, (bc + 1) * 128)
                        h_ps = ps_h.tile([128, M], F32, tag="h")
                        for j, c in enumerate(CHUNK_ORDER):
                            w = out_l * NSLOTS_AT[c]
                            nc.tensor.matmul(
                                h_ps[:, 0:w], xsq[c][:, bs], wcub_sb[:, c, 0:w],
                                start=(j == 0), stop=(j == NCHUNK - 1),
                            )

                        lin_ps = ps_small.tile([128, out_l], F32, tag="small")
                        nc.tensor.matmul(lin_ps[:], xT_sb[0:65, bs], wlin_sb[:], start=True, stop=True)
                        xmac_ps = ps_small.tile([128, NSLOT], F32, tag="small")
                        nc.tensor.matmul(xmac_ps[:], xT_sb[0:D, bs], colsel_sb[:], start=True, stop=True)
                        xmac_sb = ypool.tile([128, NSLOT], F32, tag="xmac")
                        nc.scalar.copy(xmac_sb[:], xmac_ps[:])

                        # tmp[:, :M] = h * xmac (broadcast over o); tmp[:, M:] = lin
                        tmp_sb = hpool.tile([128, M + out_l], F32, tag="tmp")
                        xmac_b = (
                            xmac_sb[:].unsqueeze(2).to_broadcast([128, NSLOT, out_l])
                        )
                        nc.vector.tensor_tensor(
                            tmp_sb[:, 0:M].rearrange("p (i o) -> p i o", i=NSLOT),
                            h_ps[:].rearrange("p (i o) -> p i o", i=NSLOT),
                            xmac_b,
                            op=MULT,
                        )
                        nc.scalar.copy(tmp_sb[:, M:], lin_ps[:])
                        nc.vector.tensor_reduce(
                            y_sb[:, bc, :],
                            tmp_sb[:].rearrange("p (i o) -> p o i", i=NSLOT + 1),
                            axis=AXIS_X, op=ADD,
                        )

                    if not last:
                        # -- phase D: AllReduce this half
                        y_bounce = dpool.tile([HB, out_l], F32, tag=f"ybounce{h}")
                        y_red = dpool.tile([HB, out_l], F32, tag=f"yred{h}")
                        nc.sync.dma_start(
                            y_bounce[:].rearrange("(bc p) o -> p bc o", p=128), y_sb[:]
                        )
                        nc.gpsimd.collective_compute(
                            "AllReduce",
                            ADD,
                            replica_groups=[list(range(NCORES))],
                            ins=[y_bounce.opt()],
                            outs=[y_red.opt()],
                        )
                        xs = xpool.tile([128, NBH, D], F32, tag=f"x{h}")
                        nc.sync.dma_start(
                            xs[:], y_red[:].rearrange("(bc p) f -> p bc f", p=128)
                        )
                        next_x[h] = xs
                    else:
                        nc.sync.dma_start(
                            out_ext.ap()[h * HB:(h + 1) * HB, :]
                            .rearrange("(bc p) o -> p bc o", p=128),
                            y_sb[:],
                        )

                if not last:
                    x_half = next_x

    nc.compile()
    return nc


# ------------------------------------------------------------------- runner --

def kernel(x, W0, b0, W1, b1, W2, b2):
    from concourse.bass_utils import run_bass_kernel_spmd

    if "nc" not in _CACHE:
        _CACHE["nc"] = _build_module()
    nc = _CACHE["nc"]

    x = np.ascontiguousarray(np.asarray(x, np.float32))
    Ws = [np.asarray(W, np.float32) for W in (W0, W1, W2)]
    bs = [np.asarray(b_, np.float32) for b_ in (b0, b1, b2)]

    wcubs, wlins = {}, {}
    for li in range(3):
        wcubs[li], wlins[li] = _prep_layer(Ws[li], bs[li], OUTS[li])

    in_maps = []
    for core in range(NCORES):
        colsel = np.zeros((D, NSLOT), np.float16)
        for s in range(NSLOT):
            colsel[8 * s + core, s] = 1.0
        m = {"x": x, "colsel": colsel}
        for li in range(3):
            m[f"wcub{li}"] = wcubs[li][core]
            m[f"wlin{li}"] = wlins[li][core]
        in_maps.append(m)

    res = run_bass_kernel_spmd(nc, in_maps, core_ids=list(range(NCORES)))
    out = np.zeros((B, OUTS[2]), np.float32)
    for core in range(NCORES):
        out += res.results[core]["out"]
    return out
